# revision 33
# baseline (speedup 1.0000x reference)
"""Trainium2 kernel for nn_PhotonicNeuralNetwork_85933705658690.

Math: the reference is a 6-qubit statevector circuit where only six
data-encoding RY gates depend on the batched scalar x (angle frequencies
3, 9, 1 and 7, 17, 1). Every amplitude is therefore a trigonometric
polynomial in x with half-integer frequencies up to 19, and the output
(p111 - p000)*2 is an exact trig polynomial of integer degree <= 38:

    out(x) = sum_{m=0}^{38} r_m * sin(m*x + phi_m)

The 39 (r_m, phi_m) pairs depend only on `params`; they are extracted on
the host with a 128-point FFT of an exact float64 simulation (cost: one
batch-128 circuit evaluation). The device kernel evaluates the harmonic
series for all 65536 x values, data-parallel over 8 NeuronCores.

Device pipeline per core (8192 elements = 5 groups of 3x512 + one 512
tail; each group packs 3 element blocks x 39 harmonics on 117
partitions):
  DMA   : broadcast each 512-block of x to 39 partitions         (in)
  DVE   : f = y - round(y), y = x*(m/2pi) + phi/2pi    [custom op:
          affine + magic-number round + frac in one pass; ACT's Sin
          table is only accurate for |arg| <= ~pi]
  ACT   : h = Sin(2*pi*f) -> float32r                   (|arg| <= pi)
  PE    : psum[3,512] = block-diag(r)^T @ h   (f32r matmul, 1 cyc/row)
  DVE/ACT: psum -> sbuf (engine-owned output slabs)
  DMA   : out (tail slab, groups 0-1 slab, groups 2-4 slab)

Scheduling notes: emission is phase-ordered with explicit same-engine
ordering deps so the Tile scheduler cannot interleave late psum copies
between compute ops; per-group tiles avoid cross-engine false deps;
the tail chunk is processed early so it does not trail the kernel.
"""

import numpy as np

# ---------------------------------------------------------------------------
# Problem constants (hardcoded per harness contract)
# ---------------------------------------------------------------------------
B = 65536
N_CORES = 8
SHARD = B // N_CORES          # 8192 per core
NH = 39                       # harmonics m = 0..38
PPACK = 3                     # element blocks packed on the partition axis
P = NH * PPACK                # 117 partitions
F = 512                       # chunk size == one PSUM bank of fp32
N_GROUPS = 5                  # 5 * (3*512) = 7680 elements
TAIL = SHARD - N_GROUPS * PPACK * F   # 512 tail elements
TWO_PI = 2.0 * np.pi
MAGIC = float(1.5 * 2.0 ** 23)  # float32 round-to-nearest-integer constant

NQ = 6

# ---------------------------------------------------------------------------
# Host-side exact circuit (float64 numpy port of the jax reference)
# ---------------------------------------------------------------------------


def _sel(q, bit, controls):
    ix = [slice(None)] * (NQ + 1)
    for cq in controls:
        ix[cq + 1] = 1
    ix[q + 1] = bit
    return tuple(ix)


def _ry(state, q, theta, controls=()):
    half = np.asarray(theta, dtype=np.float64) * 0.5
    c, s = np.cos(half), np.sin(half)
    i0, i1 = _sel(q, 0, controls), _sel(q, 1, controls)
    a0, a1 = state[i0].copy(), state[i1].copy()
    if c.ndim:
        bs = (c.shape[0],) + (1,) * (a0.ndim - 1)
        c, s = c.reshape(bs), s.reshape(bs)
    state[i0] = c * a0 - s * a1
    state[i1] = s * a0 + c * a1
    return state


def _y(state, q, controls=()):
    i0, i1 = _sel(q, 0, controls), _sel(q, 1, controls)
    a0, a1 = state[i0].copy(), state[i1].copy()
    state[i0] = -1j * a1
    state[i1] = 1j * a0
    return state


def _circuit_np(x, p):
    x = np.asarray(x, dtype=np.float64)
    p = np.asarray(p, dtype=np.float64)
    n = x.shape[0]
    st = np.zeros((n,) + (2,) * NQ, np.complex128)
    st[(slice(None),) + (0,) * NQ] = 1.0
    st = _ry(st, 0, x * 3.0); st = _ry(st, 1, x * 9.0); st = _ry(st, 2, x)
    st = _ry(st, 0, p[0]); st = _ry(st, 1, p[1]); st = _ry(st, 2, p[2])
    st = _ry(st, 3, p[3], (2,)); st = _ry(st, 3, p[4], (1,)); st = _ry(st, 3, p[5], (0,)); st = _ry(st, 3, p[6])
    st = _ry(st, 4, p[7], (2,)); st = _ry(st, 4, p[8], (1,)); st = _ry(st, 4, p[9], (0,)); st = _ry(st, 4, p[10])
    st = _ry(st, 5, p[11], (2,)); st = _ry(st, 5, p[12], (1,)); st = _ry(st, 5, p[13], (0,)); st = _ry(st, 5, p[14])
    st = _ry(st, 3, 0.5); st = _ry(st, 4, 0.5); st = _ry(st, 5, 0.5)
    st = _ry(st, 4, p[15], (3,)); st = _ry(st, 5, p[16], (3,))
    st = _ry(st, 3, p[17], (4,)); st = _ry(st, 5, p[18], (4,))
    st = _ry(st, 3, p[19], (5,)); st = _ry(st, 4, p[20], (5,))
    st = _ry(st, 3, p[21]); st = _ry(st, 4, p[22]); st = _ry(st, 5, p[23])
    st = _y(st, 0, (3, 4, 5)); st = _y(st, 1, (3, 4, 5)); st = _y(st, 2, (3, 4, 5))
    st = _ry(st, 0, x * 7.0); st = _ry(st, 1, x * 17.0); st = _ry(st, 2, x)
    st = _ry(st, 0, p[24]); st = _ry(st, 1, p[25]); st = _ry(st, 2, p[26])
    st = _ry(st, 0, -0.5); st = _ry(st, 1, -0.5); st = _ry(st, 2, -0.5)
    st = _ry(st, 1, p[27], (0,)); st = _ry(st, 2, p[28], (0,))
    st = _ry(st, 0, p[29], (1,)); st = _ry(st, 2, p[30], (1,))
    st = _ry(st, 0, p[31], (2,)); st = _ry(st, 1, p[32], (2,))
    st = _ry(st, 0, p[33]); st = _ry(st, 1, p[34]); st = _ry(st, 2, p[35])
    p000 = np.sum(np.abs(st[:, 0, 0, 0]) ** 2, axis=(1, 2, 3))
    p111 = np.sum(np.abs(st[:, 1, 1, 1]) ** 2, axis=(1, 2, 3))
    return (p111 - p000) * 2.0


def _fourier_coeffs(params, nfft=128):
    """out(x) = a0 + sum a_m cos(mx) + b_m sin(mx) = sum r_m sin(mx+phi_m)."""
    xs = TWO_PI * np.arange(nfft) / nfft
    yv = _circuit_np(xs, params)
    Fc = np.fft.rfft(yv) / nfft
    a = np.zeros(NH)
    bb = np.zeros(NH)
    a[0] = Fc[0].real
    a[1:] = 2.0 * Fc[1:NH].real
    bb[1:] = -2.0 * Fc[1:NH].imag
    r = np.hypot(a, bb)
    phi = np.arctan2(a, bb)
    return r, phi


# ---------------------------------------------------------------------------
# Custom DVE op: f = y - round(y),  y = in0*s0 + s1   (round via magic add)
# ---------------------------------------------------------------------------


def _ref_angle_frac(in0, in1, s0, s1, imm2):
    f32 = np.float32
    y = ((in0.astype(f32) * np.asarray(s0, f32)).astype(f32)
         + np.asarray(s1, f32)).astype(f32)
    bmag = (y + f32(imm2)).astype(f32)
    k = (bmag - f32(imm2)).astype(f32)
    return (y - k).astype(f32)


def _register_angle_frac():
    import concourse.dve_ops as dve_ops
    from concourse.dve_ops import DveOp
    from concourse.dve_spec import Spec, Src0, C0, C1, C2, lower
    from concourse.dve_uop import DveOpSpec

    name = "ANGLE_FRAC_ANT"
    if name in dve_ops._SUB_OPCODE_FOR_NAME:
        return next(op for op in dve_ops.OPS if op.name == name)
    Yx = Src0 * C0 + C1
    spec = Spec(body=Yx - ((Yx + C2) - C2), reference=_ref_angle_frac)
    opcode = 1 + len(dve_ops.OPS)
    assert opcode < 0x20
    shas = {}
    for ver in ("v3", "v4"):
        uops = lower(spec, ver=ver)
        shas[ver] = DveOpSpec(name=name, opcode=opcode, uops=uops,
                              rd1_en=False).sha(ver)
    op = DveOp(name, spec, subdim=False, uops_sha=shas)
    dve_ops.OPS.append(op)
    dve_ops._SUB_OPCODE_FOR_NAME[name] = opcode
    dve_ops.CUSTOM_DVE_SPECS[name] = spec
    return op


# ---------------------------------------------------------------------------
# Bass program (built once, cached)
# ---------------------------------------------------------------------------

_NC_CACHE = {}


def _build_program():
    if "nc" in _NC_CACHE:
        return _NC_CACHE["nc"]

    import concourse.bacc as bacc
    import concourse.mybir as mybir
    import concourse.tile as tile

    ANGLE_FRAC = _register_angle_frac()

    dt = mybir.dt
    AF = mybir.ActivationFunctionType

    nc = bacc.Bacc("TRN2", target_bir_lowering=False, debug=False,
                   num_devices=N_CORES)

    x_d = nc.dram_tensor("x", [1, SHARD], dt.float32, kind="ExternalInput").ap()
    # consts: col0 = m/2pi, col1 = phi/2pi, cols 2:5 = block-diag amplitudes
    c_d = nc.dram_tensor("c", [P, 5], dt.float32, kind="ExternalInput").ap()
    y_d = nc.dram_tensor("y", [1, SHARD], dt.float32, kind="ExternalOutput").ap()

    with tile.TileContext(nc) as tc:
        with (
            tc.tile_pool(name="consts", bufs=1) as cpool,
            tc.tile_pool(name="work", bufs=2) as wpool,
            tc.tile_pool(name="outp", bufs=1) as opool,
            tc.tile_pool(name="psum", bufs=1, space="PSUM") as pspool,
        ):
            from concourse.bass import _add_dep_helper
            e0 = N_GROUPS * PPACK * F

            # --- phase 0: consts first (ISAs wait on them) ---
            t_c = cpool.tile([P, 5], dt.float32, tag="c")
            nc.sync.dma_start(t_c[:], c_d[:])
            t_ampr = cpool.tile([P, PPACK], dt.float32r, tag="ampr")
            nc.vector.tensor_copy(t_ampr[:], t_c[:, 2:5])
            mp_ap = t_c[:, 0:1]
            ph_ap = t_c[:, 1:2]

            psum_g = []
            for g in range(N_GROUPS):
                pg = pspool.tile([PPACK, F], dt.float32, tag=f"ps{g}")
                psum_g.append(pg)
            psum_t = pspool.tile([1, F], dt.float32, tag="pst")
            # engine-owned output slabs: DVE owns g0,g1,tail; ACT owns g2-4
            y_dve = opool.tile([PPACK, 3 * F], dt.float32, tag="y_dve")
            y_act = opool.tile([PPACK, 3 * F], dt.float32, tag="y_act")
            f_all = opool.tile([P, N_GROUPS * F], dt.float32, tag="f_all")
            hr_all = opool.tile([P, N_GROUPS * F], dt.float32r, tag="hr_all")

            # --- phase 1: input broadcast DMAs (tail early) ---
            def xb_src(g):
                return (x_d[0, g * PPACK * F:(g + 1) * PPACK * F]
                        .rearrange("(b f) -> b f", f=F)
                        .unsqueeze(1)
                        .to_broadcast((PPACK, NH, F)))

            xb_tiles = []
            for g in range(N_GROUPS):
                t_xb = wpool.tile([P, F], dt.float32, tag=f"xb{g}")
                xb_tiles.append(t_xb)
            t_xt = wpool.tile([NH, F], dt.float32, tag="xt")

            nc.sync.dma_start(xb_tiles[0][:], xb_src(0))
            nc.gpsimd.dma_start(
                t_xt[:], x_d[:, e0:e0 + TAIL].to_broadcast((NH, TAIL)))
            for g in range(1, N_GROUPS):
                nc.sync.dma_start(xb_tiles[g][:], xb_src(g))

            # --- phase 2: range reductions (DVE), tail second ---
            t_ft = wpool.tile([NH, F], dt.float32, tag="ft")
            nc.vector._custom_dve(
                ANGLE_FRAC, out=f_all[:, 0:F], in0=xb_tiles[0][:],
                s0=mp_ap, s1=ph_ap, imm2=MAGIC)
            nc.vector._custom_dve(
                ANGLE_FRAC, out=t_ft[:], in0=t_xt[:],
                s0=mp_ap[0:NH], s1=ph_ap[0:NH], imm2=MAGIC)
            last_isa = None
            for g in range(1, N_GROUPS):
                last_isa = nc.vector._custom_dve(
                    ANGLE_FRAC, out=f_all[:, g * F:(g + 1) * F],
                    in0=xb_tiles[g][:], s0=mp_ap, s1=ph_ap, imm2=MAGIC)

            # --- phase 3: sines (ACT), per group so each chases its ISA ---
            nc.scalar.activation(hr_all[:, 0:F], f_all[:, 0:F],
                                 AF.Sin, scale=float(TWO_PI))
            t_ht = wpool.tile([NH, F], dt.float32r, tag="ht")
            nc.scalar.activation(t_ht[:], t_ft[:], AF.Sin, scale=float(TWO_PI))
            last_sin = None
            for g in range(1, N_GROUPS):
                last_sin = nc.scalar.activation(
                    hr_all[:, g * F:(g + 1) * F], f_all[:, g * F:(g + 1) * F],
                    AF.Sin, scale=float(TWO_PI))

            # --- phase 4: matmuls (PE); tail kept early ---
            mm0 = nc.tensor.matmul(psum_g[0][:], t_ampr[:], hr_all[:, 0:F],
                                   start=True, stop=True)
            mmt = nc.tensor.matmul(psum_t[:], t_ampr[0:NH, 0:1], t_ht[:],
                                   start=True, stop=True)
            _add_dep_helper(mmt.ins, mm0.ins, False, "tail mm second")
            for g in range(1, N_GROUPS):
                mm = nc.tensor.matmul(psum_g[g][:], t_ampr[:],
                                      hr_all[:, g * F:(g + 1) * F],
                                      start=True, stop=True)
                _add_dep_helper(mm.ins, mmt.ins, False, "after tail mm")

            # --- phase 5: psum -> sbuf copies (tail first so its output
            #     DMA generation clears the HWDGE queue early) ---
            cpt = nc.vector.tensor_copy(y_dve[0:1, 2 * F:3 * F], psum_t[:])
            _add_dep_helper(cpt.ins, last_isa.ins, False, "dve copies last")
            cp0 = nc.vector.tensor_copy(y_dve[:, 0:F], psum_g[0][:])
            _add_dep_helper(cp0.ins, cpt.ins, False, "tail copy first")
            cp1 = nc.vector.tensor_copy(y_dve[:, F:2 * F], psum_g[1][:])
            _add_dep_helper(cp1.ins, cp0.ins, False, "tail copy first")
            for i, g in enumerate(range(2, N_GROUPS)):
                cp = nc.scalar.copy(y_act[:, i * F:(i + 1) * F], psum_g[g][:])
                _add_dep_helper(cp.ins, last_sin.ins, False, "act copies last")

            # --- phase 6: output DMAs ---
            nc.sync.dma_start(y_d[:, e0:e0 + TAIL], y_dve[0:1, 2 * F:3 * F])
            nc.sync.dma_start(
                y_d[0, 0:2 * PPACK * F].rearrange(
                    "(g b f) -> b g f", b=PPACK, f=F),
                y_dve[:, 0:2 * F].rearrange("p (g f) -> p g f", f=F),
            )
            nc.sync.dma_start(
                y_d[0, 2 * PPACK * F:N_GROUPS * PPACK * F].rearrange(
                    "(g b f) -> b g f", b=PPACK, f=F),
                y_act[:].rearrange("p (g f) -> p g f", f=F),
            )

    nc.compile()
    _NC_CACHE["nc"] = nc
    return nc


def _make_const_block(params):
    r, phi = _fourier_coeffs(np.asarray(params, np.float64))
    m = np.arange(NH, dtype=np.float64)
    c = np.zeros((P, 5), dtype=np.float32)
    c[:, 0] = np.tile(m / TWO_PI, PPACK)
    c[:, 1] = np.tile(phi / TWO_PI, PPACK)
    for b in range(PPACK):
        c[b * NH:(b + 1) * NH, 2 + b] = r
    return c


def run_with_results(inputs, params, trace=False):
    from concourse.bass_utils import run_bass_kernel_spmd

    nc = _build_program()
    x = np.ascontiguousarray(np.asarray(inputs, np.float32)).reshape(
        N_CORES, 1, SHARD)
    c = _make_const_block(params)
    in_maps = [{"x": x[i], "c": c} for i in range(N_CORES)]
    res = run_bass_kernel_spmd(nc, in_maps, core_ids=list(range(N_CORES)),
                               trace=trace)
    out = np.concatenate(
        [res.results[i]["y"].reshape(-1) for i in range(N_CORES)])
    return out.astype(np.float32), res


def kernel(inputs, params):
    out, _ = run_with_results(inputs, params, trace=False)
    return out


# revision 35
# speedup vs baseline: 1.0296x; 1.0296x over previous
"""Trainium2 kernel for nn_PhotonicNeuralNetwork_85933705658690.

Math: the reference is a 6-qubit statevector circuit where only six
data-encoding RY gates depend on the batched scalar x (angle frequencies
3, 9, 1 and 7, 17, 1). Every amplitude is therefore a trigonometric
polynomial in x with half-integer frequencies up to 19, and the output
(p111 - p000)*2 is an exact trig polynomial of integer degree <= 38:

    out(x) = sum_{m=0}^{38} r_m * sin(m*x + phi_m)

The 39 (r_m, phi_m) pairs depend only on `params`; they are extracted on
the host with a 128-point FFT of an exact float64 simulation (cost: one
batch-128 circuit evaluation). The device kernel evaluates the harmonic
series for all 65536 x values, data-parallel over 8 NeuronCores.

Device pipeline per core (8192 elements = 5 groups of 3x512 + one 512
tail; each group packs 3 element blocks x 39 harmonics on 117
partitions):
  DMA   : broadcast each 512-block of x to 39 partitions         (in)
  DVE   : f = y - round(y), y = x*(m/2pi) + phi/2pi    [custom op:
          affine + magic-number round + frac in one pass; ACT's Sin
          table is only accurate for |arg| <= ~pi]
  ACT   : h = Sin(2*pi*f) -> float32r                   (|arg| <= pi)
  PE    : psum[3,512] = block-diag(r)^T @ h   (f32r matmul, 1 cyc/row)
  DVE/ACT: psum -> sbuf (engine-owned output slabs)
  DMA   : out (tail slab, groups 0-1 slab, groups 2-4 slab)

Scheduling notes: emission is phase-ordered with explicit same-engine
ordering deps so the Tile scheduler cannot interleave late psum copies
between compute ops; per-group tiles avoid cross-engine false deps;
the tail chunk is processed early so it does not trail the kernel.
"""

import numpy as np

# ---------------------------------------------------------------------------
# Problem constants (hardcoded per harness contract)
# ---------------------------------------------------------------------------
B = 65536
N_CORES = 8
SHARD = B // N_CORES          # 8192 per core
NH = 39                       # harmonics m = 0..38
PPACK = 3                     # element blocks packed on the partition axis
P = NH * PPACK                # 117 partitions
F = 512                       # chunk size == one PSUM bank of fp32
N_GROUPS = 5                  # 5 * (3*512) = 7680 elements
TAIL = SHARD - N_GROUPS * PPACK * F   # 512 tail elements
TWO_PI = 2.0 * np.pi
MAGIC = float(1.5 * 2.0 ** 23)  # float32 round-to-nearest-integer constant

NQ = 6

# ---------------------------------------------------------------------------
# Host-side exact circuit (float64 numpy port of the jax reference)
# ---------------------------------------------------------------------------


def _sel(q, bit, controls):
    ix = [slice(None)] * (NQ + 1)
    for cq in controls:
        ix[cq + 1] = 1
    ix[q + 1] = bit
    return tuple(ix)


def _ry(state, q, theta, controls=()):
    half = np.asarray(theta, dtype=np.float64) * 0.5
    c, s = np.cos(half), np.sin(half)
    i0, i1 = _sel(q, 0, controls), _sel(q, 1, controls)
    a0, a1 = state[i0].copy(), state[i1].copy()
    if c.ndim:
        bs = (c.shape[0],) + (1,) * (a0.ndim - 1)
        c, s = c.reshape(bs), s.reshape(bs)
    state[i0] = c * a0 - s * a1
    state[i1] = s * a0 + c * a1
    return state


def _y(state, q, controls=()):
    i0, i1 = _sel(q, 0, controls), _sel(q, 1, controls)
    a0, a1 = state[i0].copy(), state[i1].copy()
    state[i0] = -1j * a1
    state[i1] = 1j * a0
    return state


def _circuit_np(x, p):
    x = np.asarray(x, dtype=np.float64)
    p = np.asarray(p, dtype=np.float64)
    n = x.shape[0]
    st = np.zeros((n,) + (2,) * NQ, np.complex128)
    st[(slice(None),) + (0,) * NQ] = 1.0
    st = _ry(st, 0, x * 3.0); st = _ry(st, 1, x * 9.0); st = _ry(st, 2, x)
    st = _ry(st, 0, p[0]); st = _ry(st, 1, p[1]); st = _ry(st, 2, p[2])
    st = _ry(st, 3, p[3], (2,)); st = _ry(st, 3, p[4], (1,)); st = _ry(st, 3, p[5], (0,)); st = _ry(st, 3, p[6])
    st = _ry(st, 4, p[7], (2,)); st = _ry(st, 4, p[8], (1,)); st = _ry(st, 4, p[9], (0,)); st = _ry(st, 4, p[10])
    st = _ry(st, 5, p[11], (2,)); st = _ry(st, 5, p[12], (1,)); st = _ry(st, 5, p[13], (0,)); st = _ry(st, 5, p[14])
    st = _ry(st, 3, 0.5); st = _ry(st, 4, 0.5); st = _ry(st, 5, 0.5)
    st = _ry(st, 4, p[15], (3,)); st = _ry(st, 5, p[16], (3,))
    st = _ry(st, 3, p[17], (4,)); st = _ry(st, 5, p[18], (4,))
    st = _ry(st, 3, p[19], (5,)); st = _ry(st, 4, p[20], (5,))
    st = _ry(st, 3, p[21]); st = _ry(st, 4, p[22]); st = _ry(st, 5, p[23])
    st = _y(st, 0, (3, 4, 5)); st = _y(st, 1, (3, 4, 5)); st = _y(st, 2, (3, 4, 5))
    st = _ry(st, 0, x * 7.0); st = _ry(st, 1, x * 17.0); st = _ry(st, 2, x)
    st = _ry(st, 0, p[24]); st = _ry(st, 1, p[25]); st = _ry(st, 2, p[26])
    st = _ry(st, 0, -0.5); st = _ry(st, 1, -0.5); st = _ry(st, 2, -0.5)
    st = _ry(st, 1, p[27], (0,)); st = _ry(st, 2, p[28], (0,))
    st = _ry(st, 0, p[29], (1,)); st = _ry(st, 2, p[30], (1,))
    st = _ry(st, 0, p[31], (2,)); st = _ry(st, 1, p[32], (2,))
    st = _ry(st, 0, p[33]); st = _ry(st, 1, p[34]); st = _ry(st, 2, p[35])
    p000 = np.sum(np.abs(st[:, 0, 0, 0]) ** 2, axis=(1, 2, 3))
    p111 = np.sum(np.abs(st[:, 1, 1, 1]) ** 2, axis=(1, 2, 3))
    return (p111 - p000) * 2.0


def _fourier_coeffs(params, nfft=128):
    """out(x) = a0 + sum a_m cos(mx) + b_m sin(mx) = sum r_m sin(mx+phi_m)."""
    xs = TWO_PI * np.arange(nfft) / nfft
    yv = _circuit_np(xs, params)
    Fc = np.fft.rfft(yv) / nfft
    a = np.zeros(NH)
    bb = np.zeros(NH)
    a[0] = Fc[0].real
    a[1:] = 2.0 * Fc[1:NH].real
    bb[1:] = -2.0 * Fc[1:NH].imag
    r = np.hypot(a, bb)
    phi = np.arctan2(a, bb)
    return r, phi


# ---------------------------------------------------------------------------
# Custom DVE op: f = y - round(y),  y = in0*s0 + s1   (round via magic add)
# ---------------------------------------------------------------------------


def _ref_angle_frac(in0, in1, s0, s1, imm2):
    f32 = np.float32
    y = ((in0.astype(f32) * np.asarray(s0, f32)).astype(f32)
         + np.asarray(s1, f32)).astype(f32)
    bmag = (y + f32(imm2)).astype(f32)
    k = (bmag - f32(imm2)).astype(f32)
    return (y - k).astype(f32)


def _register_angle_frac():
    import concourse.dve_ops as dve_ops
    from concourse.dve_ops import DveOp
    from concourse.dve_spec import Spec, Src0, C0, C1, C2, lower
    from concourse.dve_uop import DveOpSpec

    name = "ANGLE_FRAC_ANT"
    if name in dve_ops._SUB_OPCODE_FOR_NAME:
        return next(op for op in dve_ops.OPS if op.name == name)
    Yx = Src0 * C0 + C1
    spec = Spec(body=Yx - ((Yx + C2) - C2), reference=_ref_angle_frac)
    opcode = 1 + len(dve_ops.OPS)
    assert opcode < 0x20
    shas = {}
    for ver in ("v3", "v4"):
        uops = lower(spec, ver=ver)
        shas[ver] = DveOpSpec(name=name, opcode=opcode, uops=uops,
                              rd1_en=False).sha(ver)
    op = DveOp(name, spec, subdim=False, uops_sha=shas)
    dve_ops.OPS.append(op)
    dve_ops._SUB_OPCODE_FOR_NAME[name] = opcode
    dve_ops.CUSTOM_DVE_SPECS[name] = spec
    return op


# ---------------------------------------------------------------------------
# Bass program (built once, cached)
# ---------------------------------------------------------------------------

_NC_CACHE = {}


def _build_program():
    if "nc" in _NC_CACHE:
        return _NC_CACHE["nc"]

    import concourse.bacc as bacc
    import concourse.mybir as mybir
    import concourse.tile as tile

    ANGLE_FRAC = _register_angle_frac()

    dt = mybir.dt
    AF = mybir.ActivationFunctionType

    nc = bacc.Bacc("TRN2", target_bir_lowering=False, debug=False,
                   num_devices=N_CORES)

    x_d = nc.dram_tensor("x", [1, SHARD], dt.float32, kind="ExternalInput").ap()
    # consts: col0 = m/2pi, col1 = phi/2pi, cols 2:5 = block-diag amplitudes
    c_d = nc.dram_tensor("c", [P, 5], dt.float32, kind="ExternalInput").ap()
    y_d = nc.dram_tensor("y", [1, SHARD], dt.float32, kind="ExternalOutput").ap()

    with tile.TileContext(nc) as tc:
        with (
            tc.tile_pool(name="consts", bufs=1) as cpool,
            tc.tile_pool(name="work", bufs=2) as wpool,
            tc.tile_pool(name="outp", bufs=1) as opool,
            tc.tile_pool(name="psum", bufs=1, space="PSUM") as pspool,
        ):
            from concourse.bass import _add_dep_helper
            e0 = N_GROUPS * PPACK * F

            # --- phase 0: consts first (ISAs wait on them) ---
            t_c = cpool.tile([P, 5], dt.float32, tag="c")
            nc.sync.dma_start(t_c[:], c_d[:])
            t_ampr = cpool.tile([P, PPACK], dt.float32r, tag="ampr")
            nc.vector.tensor_copy(t_ampr[:], t_c[:, 2:5])
            mp_ap = t_c[:, 0:1]
            ph_ap = t_c[:, 1:2]

            psum_A = pspool.tile([PPACK, 2 * F], dt.float32, tag="psA")
            psum_B = pspool.tile([PPACK, 2 * F], dt.float32, tag="psB")
            psum_4 = pspool.tile([PPACK, F], dt.float32, tag="ps4")
            psum_t = pspool.tile([1, F], dt.float32, tag="pst")
            psum_g = [psum_A[:, 0:F], psum_A[:, F:2 * F],
                      psum_B[:, 0:F], psum_B[:, F:2 * F], psum_4[:]]
            # engine-owned output slabs: DVE owns g0,g1,tail; ACT owns g2-4
            y_dve = opool.tile([PPACK, 3 * F], dt.float32, tag="y_dve")
            y_act = opool.tile([PPACK, 3 * F], dt.float32, tag="y_act")
            f_all = opool.tile([P, N_GROUPS * F], dt.float32, tag="f_all")
            hr_all = opool.tile([P, N_GROUPS * F], dt.float32r, tag="hr_all")

            # --- phase 1: input broadcast DMAs (tail early) ---
            def xb_src(g):
                return (x_d[0, g * PPACK * F:(g + 1) * PPACK * F]
                        .rearrange("(b f) -> b f", f=F)
                        .unsqueeze(1)
                        .to_broadcast((PPACK, NH, F)))

            xb_tiles = []
            for g in range(N_GROUPS):
                t_xb = wpool.tile([P, F], dt.float32, tag=f"xb{g}")
                xb_tiles.append(t_xb)
            t_xt = wpool.tile([NH, F], dt.float32, tag="xt")

            nc.sync.dma_start(xb_tiles[0][:], xb_src(0))
            nc.gpsimd.dma_start(
                t_xt[:], x_d[:, e0:e0 + TAIL].to_broadcast((NH, TAIL)))
            for g in range(1, N_GROUPS):
                nc.sync.dma_start(xb_tiles[g][:], xb_src(g))

            # --- phase 2: range reductions (DVE), tail second ---
            t_ft = wpool.tile([NH, F], dt.float32, tag="ft")
            nc.vector._custom_dve(
                ANGLE_FRAC, out=f_all[:, 0:F], in0=xb_tiles[0][:],
                s0=mp_ap, s1=ph_ap, imm2=MAGIC)
            nc.vector._custom_dve(
                ANGLE_FRAC, out=t_ft[:], in0=t_xt[:],
                s0=mp_ap[0:NH], s1=ph_ap[0:NH], imm2=MAGIC)
            last_isa = None
            for g in range(1, N_GROUPS):
                last_isa = nc.vector._custom_dve(
                    ANGLE_FRAC, out=f_all[:, g * F:(g + 1) * F],
                    in0=xb_tiles[g][:], s0=mp_ap, s1=ph_ap, imm2=MAGIC)

            # --- phase 3: sines (ACT), per group so each chases its ISA ---
            nc.scalar.activation(hr_all[:, 0:F], f_all[:, 0:F],
                                 AF.Sin, scale=float(TWO_PI))
            t_ht = wpool.tile([NH, F], dt.float32r, tag="ht")
            nc.scalar.activation(t_ht[:], t_ft[:], AF.Sin, scale=float(TWO_PI))
            last_sin = None
            for g in range(1, N_GROUPS):
                last_sin = nc.scalar.activation(
                    hr_all[:, g * F:(g + 1) * F], f_all[:, g * F:(g + 1) * F],
                    AF.Sin, scale=float(TWO_PI))

            # --- phase 4: matmuls (PE); tail kept early ---
            mm0 = nc.tensor.matmul(psum_g[0], t_ampr[:], hr_all[:, 0:F],
                                   start=True, stop=True)
            mmt = nc.tensor.matmul(psum_t[:], t_ampr[0:NH, 0:1], t_ht[:],
                                   start=True, stop=True)
            _add_dep_helper(mmt.ins, mm0.ins, False, "tail mm second")
            for g in range(1, N_GROUPS):
                mm = nc.tensor.matmul(psum_g[g], t_ampr[:],
                                      hr_all[:, g * F:(g + 1) * F],
                                      start=True, stop=True)
                _add_dep_helper(mm.ins, mmt.ins, False, "after tail mm")

            # --- phase 5: psum -> sbuf copies (tail first so its output
            #     DMA generation clears the HWDGE queue early) ---
            cpt = nc.vector.tensor_copy(y_dve[0:1, 2 * F:3 * F], psum_t[:])
            _add_dep_helper(cpt.ins, last_isa.ins, False, "dve copies last")
            cpA = nc.vector.tensor_copy(y_dve[:, 0:2 * F], psum_A[:])
            _add_dep_helper(cpA.ins, cpt.ins, False, "tail copy first")
            cpB = nc.scalar.copy(y_act[:, 0:2 * F], psum_B[:])
            _add_dep_helper(cpB.ins, last_sin.ins, False, "act copies last")
            cp4 = nc.scalar.copy(y_act[:, 2 * F:3 * F], psum_4[:])
            _add_dep_helper(cp4.ins, cpB.ins, False, "g4 copy last")

            # --- phase 6: output DMAs ---
            nc.sync.dma_start(y_d[:, e0:e0 + TAIL], y_dve[0:1, 2 * F:3 * F])
            nc.gpsimd.dma_start(
                y_d[0, 0:2 * PPACK * F].rearrange(
                    "(g b f) -> b g f", b=PPACK, f=F),
                y_dve[:, 0:2 * F].rearrange("p (g f) -> p g f", f=F),
            )
            nc.sync.dma_start(
                y_d[0, 2 * PPACK * F:N_GROUPS * PPACK * F].rearrange(
                    "(g b f) -> b g f", b=PPACK, f=F),
                y_act[:].rearrange("p (g f) -> p g f", f=F),
            )

    nc.compile()
    _NC_CACHE["nc"] = nc
    return nc


def _make_const_block(params):
    r, phi = _fourier_coeffs(np.asarray(params, np.float64))
    m = np.arange(NH, dtype=np.float64)
    c = np.zeros((P, 5), dtype=np.float32)
    c[:, 0] = np.tile(m / TWO_PI, PPACK)
    c[:, 1] = np.tile(phi / TWO_PI, PPACK)
    for b in range(PPACK):
        c[b * NH:(b + 1) * NH, 2 + b] = r
    return c


def run_with_results(inputs, params, trace=False):
    from concourse.bass_utils import run_bass_kernel_spmd

    nc = _build_program()
    x = np.ascontiguousarray(np.asarray(inputs, np.float32)).reshape(
        N_CORES, 1, SHARD)
    c = _make_const_block(params)
    in_maps = [{"x": x[i], "c": c} for i in range(N_CORES)]
    res = run_bass_kernel_spmd(nc, in_maps, core_ids=list(range(N_CORES)),
                               trace=trace)
    out = np.concatenate(
        [res.results[i]["y"].reshape(-1) for i in range(N_CORES)])
    return out.astype(np.float32), res


def kernel(inputs, params):
    out, _ = run_with_results(inputs, params, trace=False)
    return out
